# revision 3
# baseline (speedup 1.0000x reference)
"""Trainium2 Bass kernel for nn_Attention_layer_attention_logits.

Reference computation (B=64, C=8, Lq=128, Lk=128, D=512):
    q = query @ wq.T ; k = key @ wk.T ; v = key @ wv.T
    scores = (q @ k.T) / sqrt(D)            # [B, C, Lq, Lk]
    scores[pad] = -1e9
    sv = max over Lq                        # [B, C, Lk]
    enhanced = sv[..., None] * v.sum(Lk)    # rank-1 rows
    out = layernorm(enhanced)

Algebraic restructure (validated to rel-err ~1e-6 vs reference in fp32):
    scores = scale * query @ (wq.T @ wk) @ key.T    -> fold wq/wk into M,
             only q2 = query @ M is projected, key used directly.
    sv row l needs only max_q of q2[b] . key[b,c,l]  -> PE matmul on
             PE-transposed key, DVE reduce_max along the free dim.
    v.sum(Lk) = (key.sum(Lk)) @ wv.T                -> keysum fused into the
             mandatory PSUM->SBUF evacuation of keyT via accum_out (fp32).
    layernorm of f*u (rank-1): per-(b,c) stats of u only;
             f = sv / sqrt(sv^2 * var_u + eps); out = f*(u - mean)*gamma + beta.

Precision tiers: f = sv/sqrt(sv^2 var+eps) ~= sign(sv)/sqrt(var) -- it depends
on sv only through its sign (sv^2 var >> eps and |sv| = max of 128 ~N(0,1) is
never near 0), so the whole scores path (M, q2, scores) runs in bf16/fp32r on
the PE at full rate. The keysum/vsum/layernorm path stays fp32.

Sharding: data-parallel over batch B across 8 cores (8 batches each), weights
replicated. kernel() takes FULL inputs and returns the FULL output.
"""

import os
import numpy as np

# Problem dims (hardcoded per the self-contained-kernel contract)
B, C, LQ, LK, D = 64, 8, 128, 128, 512
N_CORES = 8
B_LOC = B // N_CORES          # 8 batches per core
NBC = B_LOC * C               # 64 (b,c) pairs per core
GB = 2                        # batches per group (group tail granularity)
NG = B_LOC // GB              # 4 groups
GBC = GB * C                  # 16 bc per group
DC = D // 128                 # 4 contraction chunks
EPS = 1e-5

# finals mode: "dma" = SBUF partition-broadcast DMA + DVE multiply
#              "pe"  = exact fp32 K=16 one-hot outer-product matmul
#              "pet" = same matmul in PE transpose-mode (2 cyc/row)
UB_MODE = os.environ.get("BASS_KERNEL_UB_MODE", "pe")
BCAST_BATCH = int(os.environ.get("BASS_KERNEL_BCAST_BATCH", "1"))
QW_TAG = "bf16"
REPEAT = int(os.environ.get("BASS_KERNEL_REPEAT", "1"))

_CACHE = {}
LAST_RESULTS = None
TRACE = bool(int(os.environ.get("BASS_KERNEL_TRACE", "0")))


def _build(beta_nonzero: bool, scale: float):
    from contextlib import ExitStack

    import concourse.bacc as bacc
    import concourse.bass as bass
    import concourse.tile as tile
    import concourse.mybir as mybir

    f32 = mybir.dt.float32
    f32r = mybir.dt.float32r
    bf16 = mybir.dt.bfloat16
    i32 = mybir.dt.int32
    Alu = mybir.AluOpType
    Act = mybir.ActivationFunctionType
    X = mybir.AxisListType.X

    QW = 128                   # scores moving width (bf16: 1 cyc/row any N)

    nc = bacc.Bacc(
        "TRN2", target_bir_lowering=False, debug=False,
        enable_asserts=False, num_devices=N_CORES,
    )

    query_d = nc.dram_tensor("query", [B_LOC, LQ, D], f32, kind="ExternalInput").ap()
    key_d = nc.dram_tensor("key", [B_LOC, C, LK, D], f32, kind="ExternalInput").ap()
    kpm_d = nc.dram_tensor("kpm", [C, LK], i32, kind="ExternalInput").ap()
    wq_d = nc.dram_tensor("wq", [D, D], f32, kind="ExternalInput").ap()
    wk_d = nc.dram_tensor("wk", [D, D], f32, kind="ExternalInput").ap()
    wv_d = nc.dram_tensor("wv", [D, D], f32, kind="ExternalInput").ap()
    gamma_d = nc.dram_tensor("gamma", [D], f32, kind="ExternalInput").ap()
    beta_d = nc.dram_tensor("beta", [D], f32, kind="ExternalInput").ap()
    ident_d = nc.dram_tensor("ident", [128, 128], f32, kind="ExternalInput").ap()
    out_d = nc.dram_tensor("out", [B_LOC, C, LK, D], f32, kind="ExternalOutput").ap()

    with tile.TileContext(nc) as tc, ExitStack() as ctx:
        pers = ctx.enter_context(tc.tile_pool(name="pers", bufs=1))
        trps = ctx.enter_context(tc.tile_pool(name="trps", bufs=2, space="PSUM"))
        msps = ctx.enter_context(tc.tile_pool(name="msps", bufs=2, space="PSUM"))
        scps = ctx.enter_context(tc.tile_pool(name="scps", bufs=2, space="PSUM"))
        stage = ctx.enter_context(tc.tile_pool(name="stage", bufs=6))
        keyt = ctx.enter_context(tc.tile_pool(name="keyt", bufs=6))
        grp = ctx.enter_context(tc.tile_pool(name="grp", bufs=2))
        outp = ctx.enter_context(tc.tile_pool(name="outp", bufs=2))
        if UB_MODE == "dma":
            ubp = ctx.enter_context(tc.tile_pool(name="ubp", bufs=2))

        # ---- persistent tiles ----
        ident = pers.tile([128, 128], f32, tag="ident")
        nc.scalar.dma_start(out=ident, in_=ident_d)
        identb = pers.tile([128, 128], bf16, tag="identb")
        nc.vector.tensor_copy(out=identb, in_=ident)
        q2t = pers.tile([128, DC, B_LOC, QW], bf16, tag="q2t")     # [d',dc,b,q]
        wvt = pers.tile([128, DC, DC, 128], f32, tag="wvt")        # [d',dpc,ec,e']
        ks = pers.tile([128, DC, NBC], f32, tag="ks")              # keysum^T
        sv = pers.tile([128, NBC], f32, tag="sv")                  # max_q scores
        om8 = pers.tile([128, C], f32, tag="om8")                  # scale*(1-mask)
        nm8 = pers.tile([128, C], f32, tag="nm8")                  # -1e9*mask
        eps1 = pers.tile([128, 1], f32, tag="eps1")
        nc.vector.memset(eps1, EPS)
        zero1 = pers.tile([128, 1], f32, tag="zero1")
        nc.vector.memset(zero1, 0.0)
        ones1 = pers.tile([1, 128], f32, tag="ones1")
        nc.vector.memset(ones1, 1.0)
        gamb = pers.tile([GBC, D], f32, tag="gamb")
        nc.scalar.dma_start(
            out=gamb,
            in_=bass.AP(tensor=gamma_d.tensor, offset=gamma_d.offset,
                        ap=[[0, GBC]] + gamma_d.ap),
        )
        if beta_nonzero:
            betb = pers.tile([128, D], f32, tag="betb")
            nc.scalar.dma_start(
                out=betb,
                in_=bass.AP(tensor=beta_d.tensor, offset=beta_d.offset,
                            ap=[[0, 128]] + beta_d.ap),
            )

        # ========== phase A: weights / query prep (bf16 fast path) ==========
        with tc.tile_pool(name="pha", bufs=1) as pha:
            # bf16 casts happen inside the SWDGE DMA (gpsimd): sign-robust path
            qnat = pha.tile([128, B_LOC, D], bf16, tag="qnat")
            nc.gpsimd.dma_start(out=qnat, in_=query_d.rearrange("b q d -> q b d"))
            wqs = pha.tile([128, DC, D], bf16, tag="wqs")
            nc.gpsimd.dma_start(out=wqs, in_=wq_d.rearrange("(ec p) d -> p ec d", p=128))
            wks = pha.tile([128, DC, D], bf16, tag="wks")
            nc.gpsimd.dma_start(out=wks, in_=wk_d.rearrange("(ec p) d -> p ec d", p=128))
            wvs = pha.tile([128, DC, D], f32, tag="wvs")
            nc.scalar.dma_start(out=wvs, in_=wv_d.rearrange("(ec p) d -> p ec d", p=128))

            # M = wq.T @ wk (bf16 inputs, fp32 accum) -> msb bf16
            msb = pha.tile([128, DC, DC, 128], bf16, tag="msb")
            for dc in range(DC):
                mp = msps.tile([128, D], f32, tag="ms")
                for ec in range(DC):
                    nc.tensor.matmul(
                        mp, wqs[:, ec, dc * 128:(dc + 1) * 128], wks[:, ec, :],
                        start=(ec == 0), stop=(ec == DC - 1))
                nc.scalar.copy(
                    out=msb[:, dc, :, :],
                    in_=mp.rearrange("p (a b) -> p a b", a=DC),
                )

            # wv^T -> wvt (exact fp32 transposes)
            for dpc in range(DC):
                tp = trps.tile([128, DC, 128], f32, tag="tp")
                for ec in range(DC):
                    nc.tensor.transpose(
                        tp[:, ec, :], wvs[:, ec, dpc * 128:(dpc + 1) * 128], ident
                    )
                nc.scalar.copy(out=wvt[:, dpc, :, :], in_=tp)

            # query^T (bf16 transposes)
            qt = pha.tile([128, DC, B_LOC, 128], bf16, tag="qt")
            for b in range(B_LOC):
                tpb = trps.tile([128, DC, 128], bf16, tag="tp")
                for dc in range(DC):
                    nc.tensor.transpose(
                        tpb[:, dc, :], qnat[:, b, dc * 128:(dc + 1) * 128], identb
                    )
                nc.scalar.copy(out=qt[:, :, b, :], in_=tpb)

            # q2^T = M-contraction with query^T -> q2t (f32r, duplicated q)
            for dpc in range(DC):
                for h in range(2):
                    qp = msps.tile([128, D], f32, tag="ms")
                    for dc in range(DC):
                        nc.tensor.matmul(
                            qp, msb[:, dc, dpc, :],
                            qt[:, dc, h * 4:h * 4 + 4, :].rearrange("p a b -> p (a b)"),
                            start=(dc == 0), stop=(dc == DC - 1))
                    nc.scalar.copy(
                        out=q2t[:, dpc, h * 4:h * 4 + 4, :],
                        in_=qp.rearrange("p (a b) -> p a b", a=4))

            # mask -> om8/nm8 (transposed to [l, c])
            mraw = pha.tile([C, LK], i32, tag="mraw")
            nc.scalar.dma_start(out=mraw, in_=kpm_d)
            mf = pha.tile([C, LK], f32, tag="mf")
            nc.vector.tensor_copy(out=mf, in_=mraw)
            mtp = msps.tile([128, C], f32, tag="ms")
            nc.tensor.transpose(mtp, mf, ident[0:C, 0:C])
            m8 = pha.tile([128, C], f32, tag="m8")
            nc.vector.tensor_copy(out=m8, in_=mtp)
            nc.vector.tensor_scalar(
                out=om8, in0=m8, scalar1=-scale, scalar2=scale,
                op0=Alu.mult, op1=Alu.add,
            )
            nc.vector.tensor_scalar(
                out=nm8, in0=m8, scalar1=-1e9, scalar2=None, op0=Alu.mult,
            )

        # ========================== main loop ===============================
        for _rep in range(REPEAT):
            def bc_body(b, c, knat, cj):
                bc = b * C + c
                # PE transpose of key[b,c]: [l, d] -> [d, l] (exact fp32)
                tp = trps.tile([128, DC, 128], f32, tag="tp")
                for dc in range(DC):
                    nc.tensor.transpose(
                        tp[:, dc, :], knat[:, cj, dc * 128:(dc + 1) * 128], ident
                    )
                # evacuate PSUM->SBUF (rounding to f32r), fusing exact fp32
                # keysum via accum_out
                kt = keyt.tile([128, DC, 128], bf16, tag="kt")
                for dc in range(DC):
                    if bc % 2 == 0:
                        nc.vector.tensor_scalar(
                            out=kt[:, dc, :], in0=tp[:, dc, :],
                            scalar1=0.0, scalar2=None, op0=Alu.add, op1=Alu.add,
                            accum_out=ks[:, dc, bc:bc + 1],
                        )
                    else:
                        nc.scalar.activation(
                            out=kt[:, dc, :], in_=tp[:, dc, :], func=Act.Copy,
                            accum_out=ks[:, dc, bc:bc + 1],
                        )
                # scores^T: [l, q] accumulated over d chunks (bf16 in, f32 acc)
                sp = scps.tile([128, QW], f32, tag="sp")
                for dc in range(DC):
                    nc.tensor.matmul(sp, kt[:, dc, :], q2t[:, dc, b, :],
                                     start=(dc == 0), stop=(dc == DC - 1))
                nc.vector.reduce_max(sv[:, bc:bc + 1], sp, axis=X)

            # Spread big DMAs over the 3 DGE queues (SP-HWDGE, ACT-HWDGE,
            # Pool-SWDGE): each queue drains at single-ring bandwidth, so
            # funneling all traffic through one queue serializes on HW.
            key_qs = [nc.sync, nc.scalar, nc.gpsimd]
            out_qs = [nc.gpsimd, nc.scalar, nc.sync]
            for g in range(NG):
                b0 = g * GB
                for bi in range(GB):
                    for ch in range(2):
                        slab_i = (b0 + bi) * 2 + ch
                        knat = stage.tile([128, 4, D], f32, tag="knat")
                        key_qs[slab_i % 3].dma_start(
                            out=knat,
                            in_=key_d[b0 + bi, ch * 4:(ch + 1) * 4].rearrange(
                                "c l d -> l c d"),
                        )
                        for cj in range(4):
                            bc_body(b0 + bi, ch * 4 + cj, knat, cj)

                # ---------------- group tail: vsum, stats, f ----------------
                g0 = b0 * C
                vt = msps.tile([128, DC, GBC], f32, tag="ms")
                for ec in range(DC):
                    for dc in range(DC):
                        nc.tensor.matmul(
                            vt[:, ec, :], wvt[:, dc, ec, :], ks[:, dc, g0:g0 + GBC],
                            start=(dc == 0), stop=(dc == DC - 1),
                        )
                vts = grp.tile([128, DC, GBC], f32, tag="vts")
                nc.vector.tensor_copy(out=vts, in_=vt)
                tpv = msps.tile([GBC, DC, 128], f32, tag="ms")
                for ec in range(DC):
                    nc.tensor.transpose(tpv[:, ec, :], vts[:, ec, :], ident)
                vsum = grp.tile([GBC, D], f32, tag="vsum")
                nc.scalar.copy(out=vsum, in_=tpv.rearrange("p a b -> p (a b)"))
                stats = grp.tile([GBC, 6], f32, tag="stats")
                nc.vector.bn_stats(out=stats, in_=vsum)
                mv = grp.tile([GBC, 2], f32, tag="mv")
                nc.vector.bn_aggr(out=mv, in_=stats)
                ubarg = grp.tile([GBC, D], f32, tag="ubarg")
                nc.vector.scalar_tensor_tensor(
                    out=ubarg, in0=vsum, scalar=mv[:, 0:1], in1=gamb,
                    op0=Alu.subtract, op1=Alu.mult,
                )
                # var -> [1,16] (PE transpose) -> [128,16] (PE ones-matmul)
                vtp = msps.tile([1, GBC], f32, tag="ms")
                nc.tensor.transpose(vtp, mv[:, 1:2], ident[0:GBC, 0:GBC])
                varT = grp.tile([1, GBC], f32, tag="varT")
                nc.vector.tensor_copy(out=varT, in_=vtp)
                vbp = msps.tile([128, GBC], f32, tag="ms")
                nc.tensor.matmul(vbp, ones1, varT, start=True, stop=True)
                varb = grp.tile([128, GBC], f32, tag="varb")
                nc.vector.tensor_copy(out=varb, in_=vbp)
                # f = svm / sqrt(svm^2 var + eps); svm = sv*scale*(1-m) - 1e9 m
                sv3 = sv[:, g0:g0 + GBC].rearrange("p (x y) -> p x y", x=GB)
                om_v = bass.AP(tensor=om8.tensor, offset=om8.offset,
                               ap=[om8.ap[0], [0, GB], om8.ap[1]])
                nm_v = bass.AP(tensor=nm8.tensor, offset=nm8.offset,
                               ap=[nm8.ap[0], [0, GB], nm8.ap[1]])
                svm = grp.tile([128, GB, C], f32, tag="svm")
                nc.vector.tensor_tensor(out=svm, in0=sv3, in1=om_v, op=Alu.mult)
                nc.vector.tensor_tensor(out=svm, in0=svm, in1=nm_v, op=Alu.add)
                svm2 = svm.rearrange("p x y -> p (x y)")
                s2 = grp.tile([128, GBC], f32, tag="s2")
                nc.scalar.activation(s2, svm2, Act.Square, bias=zero1[:, 0:1])
                t_ = grp.tile([128, GBC], f32, tag="t_")
                nc.vector.tensor_tensor(out=t_, in0=s2, in1=varb, op=Alu.mult)
                rt = grp.tile([128, GBC], f32, tag="rt")
                nc.scalar.activation(rt, t_, Act.Sqrt, bias=eps1[:, 0:1])
                rr = grp.tile([128, GBC], f32, tag="rr")
                nc.vector.reciprocal(out=rr, in_=rt)
                fg = grp.tile([128, GBC], f32, tag="fg")
                nc.vector.tensor_tensor(out=fg, in0=svm2, in1=rr, op=Alu.mult)

                # ---------------- finals ----------------
                if UB_MODE in ("pe", "pet", "bf"):
                    ftp = msps.tile([GBC, 128], f32, tag="ms")
                    nc.tensor.transpose(ftp, fg, ident)
                    ft = grp.tile([GBC, 128], f32 if UB_MODE != "bf" else bf16,
                                  tag="ft")
                    nc.vector.tensor_copy(out=ft, in_=ftp)
                    if UB_MODE == "bf":
                        ubarg_x = grp.tile([GBC, D], bf16, tag="ubx")
                        nc.vector.tensor_copy(out=ubarg_x, in_=ubarg)
                    else:
                        ubarg_x = ubarg
                for bi in range(GB):
                    if UB_MODE == "dma" and BCAST_BATCH:
                        uball = ubp.tile([128, C, D], f32, tag="uball")
                        rows = ubarg[bi * C:(bi + 1) * C, :]
                        nc.gpsimd.dma_start(
                            out=uball.rearrange("p i d -> i p d"),
                            in_=bass.AP(tensor=rows.tensor, offset=rows.offset,
                                        ap=[rows.ap[0], [0, 128], rows.ap[-1]]),
                        )
                    slot = outp.tile([128, C, D], f32, tag="slot")
                    for c in range(C):
                        i = bi * C + c
                        dst = slot[:, c, :]
                        fcol = fg[:, i:i + 1]
                        if UB_MODE in ("pe", "pet", "bf"):
                            ftm = grp.tile([GBC, 128],
                                           f32 if UB_MODE != "bf" else bf16,
                                           tag="ftm")
                            nc.vector.tensor_scalar(
                                out=ftm, in0=ft, scalar1=ident[0:GBC, i:i + 1],
                                scalar2=None, op0=Alu.mult,
                            )
                            up = scps.tile([128, D], f32, tag="up")
                            nc.tensor.matmul(up, ftm, ubarg_x, start=True, stop=True,
                                             is_transpose=(UB_MODE == "pet") or None)
                            if i % 2 == 0:
                                nc.vector.tensor_copy(out=dst, in_=up)
                            else:
                                nc.scalar.copy(out=dst, in_=up)
                        elif BCAST_BATCH:
                            nc.vector.tensor_scalar(
                                out=dst, in0=uball[:, c, :], scalar1=fcol,
                                scalar2=None, op0=Alu.mult,
                            )
                        else:
                            ub = ubp.tile([128, D], f32, tag="ub")
                            u_row = ubarg[i:i + 1, :]
                            nc.gpsimd.dma_start(
                                out=ub,
                                in_=bass.AP(tensor=u_row.tensor, offset=u_row.offset,
                                            ap=[u_row.ap[0], [0, 128], u_row.ap[-1]]),
                            )
                            nc.vector.tensor_scalar(
                                out=dst, in0=ub, scalar1=fcol,
                                scalar2=None, op0=Alu.mult,
                            )
                        if beta_nonzero:
                            nc.vector.tensor_tensor(
                                out=dst, in0=dst, in1=betb, op=Alu.add)
                    out_qs[(b0 + bi) % 3].dma_start(
                        out=out_d[b0 + bi].rearrange("c l d -> l c d"),
                        in_=slot,
                    )

    nc.compile()
    return nc


def _get_nc(beta_nonzero: bool, scale: float):
    key = (beta_nonzero, UB_MODE, BCAST_BATCH, QW_TAG, REPEAT)
    if key not in _CACHE:
        _CACHE[key] = _build(beta_nonzero, scale)
    return _CACHE[key]


def kernel(query, key, key_padding_mask, wq, wk, wv, ln_gamma, ln_beta):
    global LAST_RESULTS
    from concourse.bass_utils import run_bass_kernel_spmd

    query = np.ascontiguousarray(np.asarray(query, dtype=np.float32))
    key = np.ascontiguousarray(np.asarray(key, dtype=np.float32))
    kpm = np.ascontiguousarray(np.asarray(key_padding_mask).astype(np.int32))
    wq = np.ascontiguousarray(np.asarray(wq, dtype=np.float32))
    wk = np.ascontiguousarray(np.asarray(wk, dtype=np.float32))
    wv = np.ascontiguousarray(np.asarray(wv, dtype=np.float32))
    gamma = np.ascontiguousarray(np.asarray(ln_gamma, dtype=np.float32))
    beta = np.ascontiguousarray(np.asarray(ln_beta, dtype=np.float32))
    ident = np.eye(128, dtype=np.float32)

    scale = float(1.0 / np.sqrt(np.float32(D)))
    beta_nonzero = bool(np.any(beta != 0.0))
    nc = _get_nc(beta_nonzero, scale)

    in_maps = []
    for i in range(N_CORES):
        sl = slice(i * B_LOC, (i + 1) * B_LOC)
        in_maps.append({
            "query": np.ascontiguousarray(query[sl]),
            "key": np.ascontiguousarray(key[sl]),
            "kpm": kpm,
            "wq": wq, "wk": wk, "wv": wv,
            "gamma": gamma, "beta": beta,
            "ident": ident,
        })

    res = run_bass_kernel_spmd(
        nc, in_maps, core_ids=list(range(N_CORES)), trace=TRACE,
    )
    LAST_RESULTS = res
    out = np.concatenate([r["out"] for r in res.results], axis=0)
    return out.astype(np.float32)



# revision 27
# speedup vs baseline: 1.4983x; 1.4983x over previous
"""Trainium2 Bass kernel for nn_Attention_layer_attention_logits.

Reference computation (B=64, C=8, Lq=128, Lk=128, D=512):
    q = query @ wq.T ; k = key @ wk.T ; v = key @ wv.T
    scores = (q @ k.T) / sqrt(D)            # [B, C, Lq, Lk]
    scores[pad] = -1e9
    sv = max over Lq                        # [B, C, Lk]
    enhanced = sv[..., None] * v.sum(Lk)    # rank-1 rows
    out = layernorm(enhanced)

Algebraic restructure (validated to rel-err ~1e-6 vs reference in fp32):
    scores = scale * query @ (wq.T @ wk) @ key.T    -> fold wq/wk into M,
             only q2 = query @ M is projected, key used directly.
    sv row l needs only max_q of q2[b] . key[b,c,l]  -> PE matmul on
             PE-transposed key, DVE reduce_max along the free dim.
    v.sum(Lk) = (key.sum(Lk)) @ wv.T                -> keysum fused into the
             mandatory PSUM->SBUF evacuation of keyT via accum_out (fp32).
    layernorm of f*u (rank-1): per-(b,c) stats of u only;
             f = sv / sqrt(sv^2 * var_u + eps); out = f*(u - mean)*gamma + beta.

Precision tiers: f = sv/sqrt(sv^2 var+eps) ~= sign(sv)/sqrt(var) -- it depends
on sv only through its sign (sv^2 var >> eps and |sv| = max of 128 ~N(0,1) is
never near 0), so the whole scores path (M, q2, scores) runs in bf16/fp32r on
the PE at full rate. The keysum/vsum/layernorm path stays fp32.

Sharding: data-parallel over batch B across 8 cores (8 batches each), weights
replicated. kernel() takes FULL inputs and returns the FULL output.
"""

import os
import numpy as np

# Problem dims (hardcoded per the self-contained-kernel contract)
B, C, LQ, LK, D = 64, 8, 128, 128, 512
N_CORES = 8
B_LOC = B // N_CORES          # 8 batches per core
NBC = B_LOC * C               # 64 (b,c) pairs per core
GB = 4                        # batches per group (group tail granularity)
NG = B_LOC // GB              # 2 groups
GBC = GB * C                  # 32 bc per group
DC = D // 128                 # 4 contraction chunks
EPS = 1e-5

# finals mode: "dma" = SBUF partition-broadcast DMA + DVE multiply
#              "pe"  = f32r K=GBC one-hot outer-product matmul
#              "pet" = same matmul in PE transpose-mode (2 cyc/row)
UB_MODE = os.environ.get("BASS_KERNEL_UB_MODE", "pe")
# key-slab load path: "gather" = dma_gather on SWDGE queues 1-3 (6-queue DMA
# spread total), "plain" = round-robin plain DMAs over the 3 DGE queues
KEYQ = os.environ.get("BASS_KERNEL_KEYQ", "gather")
BCAST_BATCH = int(os.environ.get("BASS_KERNEL_BCAST_BATCH", "1"))
QW_TAG = "bf16"
REPEAT = int(os.environ.get("BASS_KERNEL_REPEAT", "1"))

_CACHE = {}
LAST_RESULTS = None
TRACE = bool(int(os.environ.get("BASS_KERNEL_TRACE", "0")))


def _build(beta_nonzero: bool, scale: float):
    from contextlib import ExitStack

    import concourse.bacc as bacc
    import concourse.bass as bass
    import concourse.tile as tile
    import concourse.mybir as mybir

    f32 = mybir.dt.float32
    f32r = mybir.dt.float32r
    bf16 = mybir.dt.bfloat16
    i32 = mybir.dt.int32
    Alu = mybir.AluOpType
    Act = mybir.ActivationFunctionType
    X = mybir.AxisListType.X

    QW = 128                   # scores moving width (bf16: 1 cyc/row any N)

    nc = bacc.Bacc(
        "TRN2", target_bir_lowering=False, debug=False,
        enable_asserts=False, num_devices=N_CORES,
        num_swdge_queues=4 if KEYQ == "gather" else 1,
    )

    query_d = nc.dram_tensor("query", [B_LOC, LQ, D], f32, kind="ExternalInput").ap()
    key_d = nc.dram_tensor("key", [B_LOC, C, LK, D], f32, kind="ExternalInput").ap()
    kpm_d = nc.dram_tensor("kpm", [C, LK], i32, kind="ExternalInput").ap()
    wq_d = nc.dram_tensor("wq", [D, D], f32, kind="ExternalInput").ap()
    wk_d = nc.dram_tensor("wk", [D, D], f32, kind="ExternalInput").ap()
    wv_d = nc.dram_tensor("wv", [D, D], f32, kind="ExternalInput").ap()
    gamma_d = nc.dram_tensor("gamma", [D], f32, kind="ExternalInput").ap()
    beta_d = nc.dram_tensor("beta", [D], f32, kind="ExternalInput").ap()
    ident_d = nc.dram_tensor("ident", [128, 128], f32, kind="ExternalInput").ap()
    if KEYQ == "gather":
        i16 = mybir.dt.int16
        gidx_d = nc.dram_tensor("gidx", [128, 2, 32], i16,
                                kind="ExternalInput").ap()
    out_d = nc.dram_tensor("out", [B_LOC, C, LK, D], f32, kind="ExternalOutput").ap()

    with tile.TileContext(nc) as tc, ExitStack() as ctx:
        pers = ctx.enter_context(tc.tile_pool(name="pers", bufs=1))
        trps = ctx.enter_context(tc.tile_pool(name="trps", bufs=2, space="PSUM"))
        msps = ctx.enter_context(tc.tile_pool(name="msps", bufs=2, space="PSUM"))
        scps = ctx.enter_context(tc.tile_pool(name="scps", bufs=2, space="PSUM"))
        stage = ctx.enter_context(tc.tile_pool(name="stage", bufs=6))
        keyt = ctx.enter_context(tc.tile_pool(name="keyt", bufs=6))
        grp = ctx.enter_context(tc.tile_pool(name="grp", bufs=2))
        outp = ctx.enter_context(tc.tile_pool(name="outp", bufs=2))
        if UB_MODE == "dma":
            ubp = ctx.enter_context(tc.tile_pool(name="ubp", bufs=2))

        # ---- persistent tiles ----
        ident = pers.tile([128, 128], f32, tag="ident")
        nc.scalar.dma_start(out=ident, in_=ident_d)
        if KEYQ == "gather":
            from concourse import library_config
            nc.gpsimd.load_library(library_config.mlp)
            gidx = pers.tile([128, 2, 32], mybir.dt.int16, tag="gidx")
            nc.scalar.dma_start(out=gidx, in_=gidx_d)
        identb = pers.tile([128, 128], bf16, tag="identb")
        nc.vector.tensor_copy(out=identb, in_=ident)
        q2t = pers.tile([128, DC, B_LOC, QW], bf16, tag="q2t")     # [d',dc,b,q]
        wvt = pers.tile([128, DC, DC, 128], f32, tag="wvt")        # [d',dpc,ec,e']
        ks = pers.tile([128, DC, NBC], f32, tag="ks")              # keysum^T
        sv = pers.tile([128, NBC], f32, tag="sv")                  # max_q scores
        om8 = pers.tile([128, C], f32, tag="om8")                  # scale*(1-mask)
        nm8 = pers.tile([128, C], f32, tag="nm8")                  # -1e9*mask
        eps1 = pers.tile([128, 1], f32, tag="eps1")
        nc.vector.memset(eps1, EPS)
        zero1 = pers.tile([128, 1], f32, tag="zero1")
        nc.vector.memset(zero1, 0.0)
        ones1 = pers.tile([1, 128], f32, tag="ones1")
        nc.vector.memset(ones1, 1.0)
        gamb = pers.tile([GBC, D], f32, tag="gamb")
        nc.scalar.dma_start(
            out=gamb,
            in_=bass.AP(tensor=gamma_d.tensor, offset=gamma_d.offset,
                        ap=[[0, GBC]] + gamma_d.ap),
        )
        if beta_nonzero:
            betb = pers.tile([128, D], f32, tag="betb")
            nc.scalar.dma_start(
                out=betb,
                in_=bass.AP(tensor=beta_d.tensor, offset=beta_d.offset,
                            ap=[[0, 128]] + beta_d.ap),
            )

        # ========== phase A: weights / query prep (bf16 fast path) ==========
        with tc.tile_pool(name="pha", bufs=1) as pha:
            # bf16 casts happen inside the SWDGE DMA (gpsimd): sign-robust path
            qnat = pha.tile([128, B_LOC, D], bf16, tag="qnat")
            nc.gpsimd.dma_start(out=qnat, in_=query_d.rearrange("b q d -> q b d"))
            wqs = pha.tile([128, DC, D], bf16, tag="wqs")
            nc.gpsimd.dma_start(out=wqs, in_=wq_d.rearrange("(ec p) d -> p ec d", p=128))
            wks = pha.tile([128, DC, D], bf16, tag="wks")
            nc.gpsimd.dma_start(out=wks, in_=wk_d.rearrange("(ec p) d -> p ec d", p=128))
            wvs = pha.tile([128, DC, D], f32, tag="wvs")
            nc.scalar.dma_start(out=wvs, in_=wv_d.rearrange("(ec p) d -> p ec d", p=128))

            # M = wq.T @ wk (bf16 inputs, fp32 accum) -> msb bf16
            msb = pha.tile([128, DC, DC, 128], bf16, tag="msb")
            for dc in range(DC):
                mp = msps.tile([128, D], f32, tag="ms")
                for ec in range(DC):
                    nc.tensor.matmul(
                        mp, wqs[:, ec, dc * 128:(dc + 1) * 128], wks[:, ec, :],
                        start=(ec == 0), stop=(ec == DC - 1))
                nc.scalar.copy(
                    out=msb[:, dc, :, :],
                    in_=mp.rearrange("p (a b) -> p a b", a=DC),
                )

            # wv^T -> wvt (exact fp32 transposes)
            for dpc in range(DC):
                tp = trps.tile([128, DC, 128], f32, tag="tp")
                for ec in range(DC):
                    nc.tensor.transpose(
                        tp[:, ec, :], wvs[:, ec, dpc * 128:(dpc + 1) * 128], ident
                    )
                nc.scalar.copy(out=wvt[:, dpc, :, :], in_=tp)

            # query^T (bf16 transposes)
            qt = pha.tile([128, DC, B_LOC, 128], bf16, tag="qt")
            for b in range(B_LOC):
                tpb = trps.tile([128, DC, 128], bf16, tag="tp")
                for dc in range(DC):
                    nc.tensor.transpose(
                        tpb[:, dc, :], qnat[:, b, dc * 128:(dc + 1) * 128], identb
                    )
                nc.scalar.copy(out=qt[:, :, b, :], in_=tpb)

            # q2^T = M-contraction with query^T -> q2t (f32r, duplicated q)
            for dpc in range(DC):
                for h in range(2):
                    qp = msps.tile([128, D], f32, tag="ms")
                    for dc in range(DC):
                        nc.tensor.matmul(
                            qp, msb[:, dc, dpc, :],
                            qt[:, dc, h * 4:h * 4 + 4, :].rearrange("p a b -> p (a b)"),
                            start=(dc == 0), stop=(dc == DC - 1))
                    nc.scalar.copy(
                        out=q2t[:, dpc, h * 4:h * 4 + 4, :],
                        in_=qp.rearrange("p (a b) -> p a b", a=4))

            # mask -> om8/nm8 (transposed to [l, c])
            mraw = pha.tile([C, LK], i32, tag="mraw")
            nc.scalar.dma_start(out=mraw, in_=kpm_d)
            mf = pha.tile([C, LK], f32, tag="mf")
            nc.vector.tensor_copy(out=mf, in_=mraw)
            mtp = msps.tile([128, C], f32, tag="ms")
            nc.tensor.transpose(mtp, mf, ident[0:C, 0:C])
            m8 = pha.tile([128, C], f32, tag="m8")
            nc.vector.tensor_copy(out=m8, in_=mtp)
            nc.vector.tensor_scalar(
                out=om8, in0=m8, scalar1=-scale, scalar2=scale,
                op0=Alu.mult, op1=Alu.add,
            )
            nc.vector.tensor_scalar(
                out=nm8, in0=m8, scalar1=-1e9, scalar2=None, op0=Alu.mult,
            )

        # ========================== main loop ===============================
        for _rep in range(REPEAT):
            def bc_body(b, c, knat, cj, sp):
                bc = b * C + c
                # PE transpose of key[b,c]: [l, d] -> [d, l] (exact fp32)
                tp = trps.tile([128, DC, 128], f32, tag="tp")
                for dc in range(DC):
                    nc.tensor.transpose(
                        tp[:, dc, :], knat[:, cj, dc * 128:(dc + 1) * 128], ident
                    )
                # evacuate PSUM->SBUF in one strided copy (bf16 for the
                # scores matmul), alternating engines; keysum via one DVE
                # reduce over the bf16 kt (rel ~2e-3, well under tolerance)
                kt = keyt.tile([128, DC, 128], bf16, tag="kt")
                nc.scalar.copy(
                    out=kt.rearrange("p a b -> p (a b)"),
                    in_=tp.rearrange("p a b -> p (a b)"),
                )
                nc.vector.reduce_sum(ks[:, :, bc:bc + 1], kt, axis=X)
                # scores^T: [l, q] accumulated over d chunks (bf16 in, f32 acc)
                for dc in range(DC):
                    nc.tensor.matmul(sp[:, cj, :], kt[:, dc, :], q2t[:, dc, b, :],
                                     start=(dc == 0), stop=(dc == DC - 1))

            # Spread big DMAs over all DGE queues: each queue drains at
            # single-ring bandwidth, so funneling all traffic through one
            # queue serializes on HW. "gather" mode puts key loads on SWDGE
            # queues 1-3 (dma_gather with consecutive row indices == the
            # l-major slab load), leaving SP/ACT-HWDGE + SWDGE q0 for stores.
            key_qs = [nc.sync, nc.scalar, nc.gpsimd]
            out_qs = [nc.gpsimd, nc.scalar, nc.sync]
            for g in range(NG):
                b0 = g * GB
                for bi in range(GB):
                    for ch in range(2):
                        slab_i = (b0 + bi) * 2 + ch
                        knat = stage.tile([128, 4, D], f32, tag="knat")
                        if KEYQ == "gather":
                            nc.gpsimd.dma_gather(
                                knat,
                                key_d[b0 + bi].rearrange("c l d -> (c l) d"),
                                gidx[:, ch, :], LK * 4, LK * 4, D,
                                queue_num=1 + slab_i % 3,
                            )
                        else:
                            key_qs[slab_i % 3].dma_start(
                                out=knat,
                                in_=key_d[b0 + bi, ch * 4:(ch + 1) * 4
                                          ].rearrange("c l d -> l c d"),
                            )
                        # one scores-PSUM bank + one reduce_max per 4 (b,c)
                        sp = scps.tile([128, 4, QW], f32, tag="sp")
                        for cj in range(4):
                            bc_body(b0 + bi, ch * 4 + cj, knat, cj, sp)
                        bc0 = (b0 + bi) * C + ch * 4
                        nc.vector.reduce_max(sv[:, bc0:bc0 + 4], sp, axis=X)

                # ---------------- group tail: vsum, stats, f ----------------
                g0 = b0 * C
                vt = msps.tile([128, DC, GBC], f32, tag="ms")
                for ec in range(DC):
                    for dc in range(DC):
                        nc.tensor.matmul(
                            vt[:, ec, :], wvt[:, dc, ec, :], ks[:, dc, g0:g0 + GBC],
                            start=(dc == 0), stop=(dc == DC - 1),
                        )
                vts = grp.tile([128, DC, GBC], f32, tag="vts")
                nc.vector.tensor_copy(out=vts, in_=vt)
                tpv = msps.tile([GBC, DC, 128], f32, tag="ms")
                for ec in range(DC):
                    nc.tensor.transpose(tpv[:, ec, :], vts[:, ec, :], ident)
                vsum = grp.tile([GBC, D], f32, tag="vsum")
                nc.scalar.copy(out=vsum, in_=tpv.rearrange("p a b -> p (a b)"))
                stats = grp.tile([GBC, 6], f32, tag="stats")
                nc.vector.bn_stats(out=stats, in_=vsum)
                mv = grp.tile([GBC, 2], f32, tag="mv")
                nc.vector.bn_aggr(out=mv, in_=stats)
                ubarg = grp.tile([GBC, D], f32, tag="ubarg")
                nc.vector.scalar_tensor_tensor(
                    out=ubarg, in0=vsum, scalar=mv[:, 0:1], in1=gamb,
                    op0=Alu.subtract, op1=Alu.mult,
                )
                # var -> [1,16] (PE transpose) -> [128,16] (PE ones-matmul)
                vtp = msps.tile([1, GBC], f32, tag="ms")
                nc.tensor.transpose(vtp, mv[:, 1:2], ident[0:GBC, 0:GBC])
                varT = grp.tile([1, GBC], f32, tag="varT")
                nc.vector.tensor_copy(out=varT, in_=vtp)
                vbp = msps.tile([128, GBC], f32, tag="ms")
                nc.tensor.matmul(vbp, ones1, varT, start=True, stop=True)
                varb = grp.tile([128, GBC], f32, tag="varb")
                nc.vector.tensor_copy(out=varb, in_=vbp)
                # f = svm / sqrt(svm^2 var + eps); svm = sv*scale*(1-m) - 1e9 m
                sv3 = sv[:, g0:g0 + GBC].rearrange("p (x y) -> p x y", x=GB)
                om_v = bass.AP(tensor=om8.tensor, offset=om8.offset,
                               ap=[om8.ap[0], [0, GB], om8.ap[1]])
                nm_v = bass.AP(tensor=nm8.tensor, offset=nm8.offset,
                               ap=[nm8.ap[0], [0, GB], nm8.ap[1]])
                svm = grp.tile([128, GB, C], f32, tag="svm")
                nc.vector.tensor_tensor(out=svm, in0=sv3, in1=om_v, op=Alu.mult)
                nc.vector.tensor_tensor(out=svm, in0=svm, in1=nm_v, op=Alu.add)
                svm2 = svm.rearrange("p x y -> p (x y)")
                s2 = grp.tile([128, GBC], f32, tag="s2")
                nc.scalar.activation(s2, svm2, Act.Square, bias=zero1[:, 0:1])
                t_ = grp.tile([128, GBC], f32, tag="t_")
                nc.vector.tensor_tensor(out=t_, in0=s2, in1=varb, op=Alu.mult)
                rt = grp.tile([128, GBC], f32, tag="rt")
                nc.scalar.activation(rt, t_, Act.Sqrt, bias=eps1[:, 0:1])
                rr = grp.tile([128, GBC], f32, tag="rr")
                nc.vector.reciprocal(out=rr, in_=rt)
                fg = grp.tile([128, GBC], f32, tag="fg")
                nc.vector.tensor_tensor(out=fg, in0=svm2, in1=rr, op=Alu.mult)

                # ---------------- finals ----------------
                if UB_MODE in ("pe", "pet", "bf"):
                    ftp = msps.tile([GBC, 128], f32, tag="ms")
                    nc.tensor.transpose(ftp, fg, ident)
                    ft = grp.tile([GBC, 128], f32 if UB_MODE != "bf" else bf16,
                                  tag="ft")
                    nc.vector.tensor_copy(out=ft, in_=ftp)
                    if UB_MODE == "bf":
                        ubarg_x = grp.tile([GBC, D], bf16, tag="ubx")
                        nc.vector.tensor_copy(out=ubarg_x, in_=ubarg)
                    elif UB_MODE == "pe":
                        # rounded-to-f32r copy: PE runs f32r matmul at
                        # 1 cyc/row (walrus requires producer-side rounding)
                        ubarg_x = grp.tile([GBC, D], f32r, tag="ubx")
                        nc.vector.tensor_copy(out=ubarg_x, in_=ubarg)
                    else:
                        ubarg_x = ubarg
                for bi in range(GB):
                    if UB_MODE == "dma" and BCAST_BATCH:
                        uball = ubp.tile([128, C, D], f32, tag="uball")
                        rows = ubarg[bi * C:(bi + 1) * C, :]
                        nc.gpsimd.dma_start(
                            out=uball.rearrange("p i d -> i p d"),
                            in_=bass.AP(tensor=rows.tensor, offset=rows.offset,
                                        ap=[rows.ap[0], [0, 128], rows.ap[-1]]),
                        )
                    slot = outp.tile([128, C, D], f32, tag="slot")
                    for c in range(C):
                        i = bi * C + c
                        dst = slot[:, c, :]
                        fcol = fg[:, i:i + 1]
                        if UB_MODE in ("pe", "pet", "bf"):
                            ftm = grp.tile([GBC, 128],
                                           {"bf": bf16, "pe": f32r}.get(
                                               UB_MODE, f32),
                                           tag="ftm")
                            nc.vector.tensor_scalar(
                                out=ftm, in0=ft, scalar1=ident[0:GBC, i:i + 1],
                                scalar2=None, op0=Alu.mult,
                            )
                            up = scps.tile([128, D], f32, tag="up")
                            if UB_MODE == "pe":
                                # f32r: 1 cyc/row at N=512 (4x over fp32)
                                nc.tensor.matmul(up, ftm, ubarg_x,
                                                 start=True, stop=True)
                            else:
                                nc.tensor.matmul(up, ftm, ubarg_x, start=True,
                                                 stop=True,
                                                 is_transpose=(UB_MODE == "pet")
                                                 or None)
                            if i % 2 == 0:
                                nc.vector.tensor_copy(out=dst, in_=up)
                            else:
                                nc.scalar.copy(out=dst, in_=up)
                        elif BCAST_BATCH:
                            nc.vector.tensor_scalar(
                                out=dst, in0=uball[:, c, :], scalar1=fcol,
                                scalar2=None, op0=Alu.mult,
                            )
                        else:
                            ub = ubp.tile([128, D], f32, tag="ub")
                            u_row = ubarg[i:i + 1, :]
                            nc.gpsimd.dma_start(
                                out=ub,
                                in_=bass.AP(tensor=u_row.tensor, offset=u_row.offset,
                                            ap=[u_row.ap[0], [0, 128], u_row.ap[-1]]),
                            )
                            nc.vector.tensor_scalar(
                                out=dst, in0=ub, scalar1=fcol,
                                scalar2=None, op0=Alu.mult,
                            )
                        if beta_nonzero:
                            nc.vector.tensor_tensor(
                                out=dst, in0=dst, in1=betb, op=Alu.add)
                    out_qs[(b0 + bi) % 3].dma_start(
                        out=out_d[b0 + bi].rearrange("c l d -> l c d"),
                        in_=slot,
                    )

    nc.compile()
    return nc


def _gidx_host():
    """Row indices for the key-slab gather: slab ch covers rows
    ch*512..ch*512+511 of key[b] viewed [(c l), d]; idx j lives at
    [partition j%16, col j//16], replicated over the partition blocks."""
    gidx = np.zeros((128, 2, 32), np.int16)
    for ch in range(2):
        for j in range(512):
            gidx[j % 16, ch, j // 16] = ch * 512 + j
    gidx[16:] = np.tile(gidx[:16], (7, 1, 1))
    return gidx


def _get_nc(beta_nonzero: bool, scale: float):
    key = (beta_nonzero, UB_MODE, BCAST_BATCH, QW_TAG, REPEAT, KEYQ)
    if key not in _CACHE:
        _CACHE[key] = _build(beta_nonzero, scale)
    return _CACHE[key]


def kernel(query, key, key_padding_mask, wq, wk, wv, ln_gamma, ln_beta):
    global LAST_RESULTS
    from concourse.bass_utils import run_bass_kernel_spmd

    query = np.ascontiguousarray(np.asarray(query, dtype=np.float32))
    key = np.ascontiguousarray(np.asarray(key, dtype=np.float32))
    kpm = np.ascontiguousarray(np.asarray(key_padding_mask).astype(np.int32))
    wq = np.ascontiguousarray(np.asarray(wq, dtype=np.float32))
    wk = np.ascontiguousarray(np.asarray(wk, dtype=np.float32))
    wv = np.ascontiguousarray(np.asarray(wv, dtype=np.float32))
    gamma = np.ascontiguousarray(np.asarray(ln_gamma, dtype=np.float32))
    beta = np.ascontiguousarray(np.asarray(ln_beta, dtype=np.float32))
    ident = np.eye(128, dtype=np.float32)

    scale = float(1.0 / np.sqrt(np.float32(D)))
    beta_nonzero = bool(np.any(beta != 0.0))
    nc = _get_nc(beta_nonzero, scale)

    if KEYQ == "gather":
        gidx = _gidx_host()

    in_maps = []
    for i in range(N_CORES):
        sl = slice(i * B_LOC, (i + 1) * B_LOC)
        m = {
            "query": np.ascontiguousarray(query[sl]),
            "key": np.ascontiguousarray(key[sl]),
            "kpm": kpm,
            "wq": wq, "wk": wk, "wv": wv,
            "gamma": gamma, "beta": beta,
            "ident": ident,
        }
        if KEYQ == "gather":
            m["gidx"] = gidx
        in_maps.append(m)

    res = run_bass_kernel_spmd(
        nc, in_maps, core_ids=list(range(N_CORES)), trace=TRACE,
    )
    LAST_RESULTS = res
    out = np.concatenate([r["out"] for r in res.results], axis=0)
    return out.astype(np.float32)

